# revision 1
# baseline (speedup 1.0000x reference)
"""Trainium2 Bass kernel for the attention-scoring module:

    out[b, s] = softmax_s( (enc[b] @ W.T + bias) @ h[b] )

Math: the bias term contributes a constant per (b, :) row, which cancels in
the softmax, and the two contractions reassociate:

    energies[b, s] = enc[b, s, :] . v[b]   with   v[b] = h[b] @ W

Sharding: data-parallel over batch — one batch per NeuronCore (B == 8 cores).
Per core: compute v on-device (tiny PE matmuls), stream enc[b] (16 MB) through
SBUF in ~1 MB DMA chunks, do the dot-products as fused multiply+row-sum DVE
instructions, then a softmax whose max/exp/transpose mostly overlap the
stream; only a short normalization chain runs after the last element arrives.
"""

from contextlib import ExitStack

import numpy as np

import concourse.tile as tile
from concourse import bacc, mybir
from concourse.bass_utils import run_bass_kernel_spmd
from concourse.masks import make_identity

B, S, H = 8, 8192, 512
N_CORES = 8
P = 128
N_COLS = S // P  # 64 energy columns, E[p, t] = energy(s = t*128 + p)
F32 = mybir.dt.float32
ALU = mybir.AluOpType
ACTF = mybir.ActivationFunctionType
AXX = mybir.AxisListType.X

CHUNK_ROWS = [512] * 15 + [384, 128]  # ~1 MB DMA chunks, tapered tail
CHUNK_BUFS = 16
EARLY_COLS = 32  # softmax shift comes from the first 32 columns, mid-stream


def _build_kernel():
    nc = bacc.Bacc("TRN2", target_bir_lowering=False, debug=False)
    enc = nc.dram_tensor("enc", [S, H], F32, kind="ExternalInput")
    hvec = nc.dram_tensor("hvec", [1, H], F32, kind="ExternalInput")
    Wmat = nc.dram_tensor("W", [H, H], F32, kind="ExternalInput")
    out = nc.dram_tensor("out", [S], F32, kind="ExternalOutput")

    with ExitStack() as ctx:
        tc = ctx.enter_context(tile.TileContext(nc))
        consts = ctx.enter_context(tc.tile_pool(name="consts", bufs=1))
        small = ctx.enter_context(tc.tile_pool(name="small", bufs=1))
        chunks = ctx.enter_context(tc.tile_pool(name="chunks", bufs=CHUNK_BUFS))
        scratch = ctx.enter_context(tc.tile_pool(name="scratch", bufs=2))
        psum = ctx.enter_context(tc.tile_pool(name="psum", bufs=1, space="PSUM"))
        psum1 = ctx.enter_context(tc.tile_pool(name="psum1", bufs=1, space="PSUM"))

        # Constants.
        identity = consts.tile([P, P], F32)
        make_identity(nc, identity[:])
        one11 = consts.tile([1, 1], F32)
        nc.gpsimd.memset(one11[:], 1.0)
        ones_row = consts.tile([1, P], F32)
        nc.gpsimd.memset(ones_row[:], 1.0)
        neg_ones_row = consts.tile([1, P], F32)
        nc.gpsimd.memset(neg_ones_row[:], -1.0)
        ones_col = consts.tile([P, 1], F32)
        nc.gpsimd.memset(ones_col[:], 1.0)

        # ---- v = h @ W, broadcast to all 128 partitions ----
        # W rides the same HWDGE ring as — and is queued ahead of — the enc
        # chunks, in 4 k-chunk DMAs so the v matmuls pipeline with arrival.
        v_bc = psum1.tile([P, H], F32)
        v_sb = small.tile([P, H], F32)
        hrow = small.tile([1, H], F32)
        nc.sync.dma_start(hrow[:], hvec.ap())
        W_c = []
        for c in range(4):
            wchunk = small.tile([P, H], F32, tag=f"wc{c}")
            W_c.append(wchunk)
            nc.sync.dma_start(wchunk[:], Wmat.ap()[c * P : (c + 1) * P, :])

        # h transposed into k-on-partitions layout: h_k[p, c] = h[c*128 + p]
        hT_ps = psum1.tile([P, 4], F32)
        for c in range(4):
            nc.tensor.matmul(
                hT_ps[:, c : c + 1],
                hrow[:1, c * P : (c + 1) * P],
                one11[:],
                start=True,
                stop=True,
            )
        h_sb = small.tile([P, 4], F32)
        nc.scalar.copy(h_sb[:], hT_ps[:])

        # v_row[0, n] = sum_k h[k] W[k, n]
        v_row_ps = psum1.tile([1, H], F32)
        for c in range(4):
            nc.tensor.matmul(
                v_row_ps[:],
                h_sb[:, c : c + 1],
                W_c[c][:],
                start=(c == 0),
                stop=(c == 3),
            )
        v_row = small.tile([1, H], F32)
        nc.scalar.copy(v_row[:], v_row_ps[:])
        # broadcast to 128 partitions: ones[1,128].T @ v_row[1,512]
        nc.tensor.matmul(v_bc[:], ones_row[:], v_row[:], start=True, stop=True)

        # Trigger the ACT exp table load at t=0 instead of in the tail.
        dummy_act = small.tile([1, 1], F32)
        nc.scalar.activation(dummy_act[:], one11[:], ACTF.Exp, bias=0.0, scale=1.0)

        # ---- main loop: stream enc, fused multiply+reduce on DVE ----
        # Within each chunk: s = s0 + c*128 + p  ->  partition p, subtile c.
        E = small.tile([P, N_COLS], F32)

        # Softmax tiles, partly written during the stream.
        P_exp = small.tile([P, N_COLS], F32)
        rs1 = small.tile([P, 1], F32)
        rs2 = small.tile([P, 1], F32)
        negM_sb = small.tile([P, 1], F32)
        negM_ps = psum.tile([P, 1], F32, tag="colp")
        probsT_ps = psum.tile([N_COLS, P], F32, tag="outp")
        EC = EARLY_COLS

        def emit_early_chain():
            # Softmax shift from the first EC columns, computed mid-stream.
            # Any shift within ~88 of the true max keeps exp() finite, and
            # the shift cancels exactly in the final normalization.
            m_col = small.tile([P, 1], F32)
            nc.vector.tensor_reduce(m_col[:], E[:, :EC], axis=AXX, op=ALU.max)
            gmax = small.tile([1, 1], F32)
            nc.gpsimd.tensor_reduce(
                gmax[:], m_col[:], axis=mybir.AxisListType.C, op=ALU.max
            )
            # broadcast -shift to all partitions via matmul with -1s
            nc.tensor.matmul(
                negM_ps[:], neg_ones_row[:], gmax[:], start=True, stop=True
            )
            nc.scalar.copy(negM_sb[:], negM_ps[:])
            # exp + row-sum + transpose of the early columns, off critical path
            nc.scalar.activation(
                P_exp[:, :EC],
                E[:, :EC],
                ACTF.Exp,
                bias=negM_sb[:],
                scale=1.0,
                accum_out=rs1[:],
            )
            nc.tensor.transpose(probsT_ps[:EC, :], P_exp[:, :EC], identity[:])

        s0 = 0
        for k, rows in enumerate(CHUNK_ROWS):
            sub = rows // P
            ch = chunks.tile([P, sub, H], F32, tag="chunk")
            src = enc.ap()[s0 : s0 + rows, :].rearrange("(c p) h -> p c h", c=sub, p=P)
            nc.sync.dma_start(ch[:], src)
            if k == 1:
                # off the critical path: migrate v from PSUM to SBUF
                nc.scalar.copy(v_sb[:], v_bc[:])
            vin = v_bc if k < 2 else v_sb
            for j in range(sub):
                prod = scratch.tile([P, H], F32, tag="prod")
                t = s0 // P + j
                # fused multiply + free-dim sum in one DVE instruction
                nc.vector.scalar_tensor_tensor(
                    out=prod[:],
                    in0=ch[:, j, :],
                    scalar=1.0,
                    in1=vin[:],
                    op0=ALU.bypass,
                    op1=ALU.mult,
                    accum_out=E[:, t : t + 1],
                )
                if t + 1 == EC:
                    emit_early_chain()
            s0 += rows

        # ---- softmax tail: only the trailing columns remain ----
        nc.scalar.activation(
            P_exp[:, EC:],
            E[:, EC:],
            ACTF.Exp,
            bias=negM_sb[:],
            scale=1.0,
            accum_out=rs2[:],
        )
        probsT2_ps = psum.tile([N_COLS - EC, P], F32, tag="outp2")
        nc.tensor.transpose(probsT2_ps[:], P_exp[:, EC:], identity[:])
        rowsum = small.tile([P, 1], F32)
        nc.vector.tensor_add(rowsum[:], rs1[:], rs2[:])
        # total sum across partitions via matmul with ones
        S_ps = psum.tile([1, 1], F32, tag="rowp")
        nc.tensor.matmul(S_ps[:], rowsum[:], ones_col[:], start=True, stop=True)
        Sinv = small.tile([1, 1], F32)
        nc.vector.reciprocal(Sinv[:], S_ps[:])
        SinvB_ps = psum.tile([P, 1], F32, tag="colp")
        nc.tensor.matmul(SinvB_ps[:], ones_row[:], Sinv[:], start=True, stop=True)

        # scale by 1/S during the PSUM->SBUF copy, then store (row t of the
        # transposed tile is the seq range [t*128, (t+1)*128) — contiguous)
        probsT = small.tile([N_COLS, P], F32)
        nc.vector.tensor_scalar_mul(probsT[:EC, :], probsT_ps[:EC, :], SinvB_ps[:EC, :])
        nc.vector.tensor_scalar_mul(
            probsT[EC:, :], probsT2_ps[:], SinvB_ps[: N_COLS - EC, :]
        )
        nc.scalar.dma_start(out.ap().rearrange("(t p) -> t p", p=P), probsT[:])

    nc.compile()
    return nc


_NC_CACHE = {}


def kernel(hidden, encoder_outputs, W, b):
    """Full (unsharded) inputs in, full output out; 8-core SPMD inside."""
    if "nc" not in _NC_CACHE:
        _NC_CACHE["nc"] = _build_kernel()
    nc = _NC_CACHE["nc"]

    hidden = np.asarray(hidden)
    enc = np.ascontiguousarray(np.asarray(encoder_outputs, dtype=np.float32))
    Wm = np.ascontiguousarray(np.asarray(W, dtype=np.float32))
    in_maps = [
        {
            "enc": enc[c],
            "hvec": np.ascontiguousarray(hidden[0, c][None, :].astype(np.float32)),
            "W": Wm,
        }
        for c in range(N_CORES)
    ]
    res = run_bass_kernel_spmd(nc, in_maps, core_ids=list(range(N_CORES)))
    return np.stack([res.results[c]["out"] for c in range(N_CORES)], axis=0).astype(
        np.float32
    )

